# revision 12
# baseline (speedup 1.0000x reference)
"""PixelUnshuffle(s=2) + avg-pool concat kernel for Trainium2, 8 NeuronCores.

Semantics (per image):
  out[i, j, 4c + 2b + a] = images[2i + a, 2j + b, c]   for c<3, a,b in {0,1}
  out[i, j, 12]          = mean of maps[2i:2i+2, 2j:2j+2]

Sharding: pure data-parallel over the batch dim (32 images -> 4 per core).

Per-core pipeline (per image):
  - one big HWDGE DMA: images rows -> SBUF, partition p holds input rows
    4p..4p+3 (contiguous 24.6 KB per partition)
  - one DMA: maps rows -> SBUF (same row grouping)
  - 4 strided tensor_copy ops (one per (a,b) phase) spread across
    DVE/ACT/POOL engines rearrange the image data into the output layout
  - 2 pool_avg ops (horizontal then vertical) compute the 2x2 mean into
    output channel 12
  - one big DMA: SBUF -> out (26.6 KB per partition)
The op is memory-bound; all engine work hides under the ~30 MB/core of
DMA traffic.
"""

import numpy as np

import concourse.bacc as bacc
import concourse.mybir as mybir
from concourse.tile import TileContext

N_CORES = 8
B, H, W, C = 32, 512, 512, 3
S = 2
BC = B // N_CORES  # images per core
HO, WO = H // S, W // S
K = C * S * S + 1  # 13 output channels
P = 128  # SBUF partitions

_FP = mybir.dt.float32


def build_nc(bc=BC, h=H, w=W, c=C):
    """Build the SPMD Bass program for one core handling `bc` images."""
    ho, wo = h // S, w // S
    k = c * S * S + 1
    R = h // P  # input rows per partition (4 for full size)
    r = R // S  # output rows per partition (2 for full size)
    assert R % S == 0 and h % P == 0

    # Bacc (not raw Bass): its finalize() legalizes sync waits down to the
    # per-instruction caps walrus codegen enforces.
    nc = bacc.Bacc()
    images = nc.declare_dram_parameter("images", [bc, h, w, c], _FP, isOutput=False)
    maps = nc.declare_dram_parameter("maps", [bc, h, w, 1], _FP, isOutput=False)
    out = nc.declare_dram_parameter("out", [bc, ho, wo, k], _FP, isOutput=True)

    # engines for the 4 strided rearrange copies, one per (a, b) phase.
    # Keep the number of distinct engines writing out_tile small: walrus
    # codegen caps the sync-wait commands on the store DMA instruction.
    copy_engines = [nc.vector, nc.vector, nc.vector, nc.vector]

    with TileContext(nc) as tc:
        with tc.tile_pool(name="sbuf", bufs=2) as pool:
            for i in range(bc):
                img_tile = pool.tile([P, R * w * c], _FP, tag="img")
                map_tile = pool.tile([P, R * w], _FP, tag="map")
                out_tile = pool.tile([P, r * wo * k], _FP, tag="out")
                havg_tile = pool.tile([P, R * wo], _FP, tag="havg")

                # loads: partition p <- input rows R*p .. R*p+R-1 (contiguous)
                nc.sync.dma_start(
                    out=img_tile[:],
                    in_=images[i].rearrange("(p f) w c -> p (f w c)", p=P),
                )
                nc.sync.dma_start(
                    out=map_tile[:],
                    in_=maps[i].rearrange("(p f) w c -> p (f w c)", p=P),
                )

                # image rearrange: for each (a, b), copy
                #   in : rows a::2 (within partition), cols b::2, all c
                #   out: channels (2b + a)::4 of every output pixel
                v_in = img_tile[:].rearrange("p (R w c) -> p R w c", R=R, w=w)
                v_out = out_tile[:].rearrange("p (r w k) -> p r w k", r=r, w=wo)
                idx = 0
                for a in range(S):
                    for b in range(S):
                        eng = copy_engines[idx % len(copy_engines)]
                        idx += 1
                        copy_fn = getattr(eng, "tensor_copy", None) or (
                            lambda out, in_: eng.copy(out, in_)
                        )
                        copy_fn(
                            out=v_out[:, :, :, 2 * b + a : k - 1 : 4],
                            in_=v_in[:, a::2, b::2, :],
                        )

                # maps 2x2 mean -> channel 12: horizontal add, vertical add,
                # then scale-by-0.25 into the output tile (all on gpsimd)
                m3 = map_tile[:].rearrange("p (R w) -> p R w", R=R)
                h_out = havg_tile[:].rearrange("p (R j) -> p R j", R=R)
                nc.gpsimd.tensor_add(
                    out=h_out, in0=m3[:, :, 0::2], in1=m3[:, :, 1::2]
                )
                h4 = havg_tile[:].rearrange("p (R j) -> p R j", R=R)
                vavg_tile = pool.tile([P, r * wo], _FP, tag="vavg")
                v4 = vavg_tile[:].rearrange("p (r j) -> p r j", r=r)
                nc.gpsimd.tensor_add(
                    out=v4, in0=h4[:, 0::2, :], in1=h4[:, 1::2, :]
                )
                # final scaled write on vector so out_tile has a single
                # producer engine (keeps the store DMA at one sync wait)
                nc.vector.tensor_scalar_mul(v_out[:, :, :, k - 1], v4, 0.25)

                # store: partition p -> output rows r*p .. r*p+r-1
                nc.sync.dma_start(
                    out=out[i].rearrange("(p f) w k -> p (f w k)", p=P),
                    in_=out_tile[:],
                )
    nc.finalize()
    return nc


_CACHED_NC = None


def kernel(**inputs: np.ndarray) -> np.ndarray:
    from concourse.bass_utils import run_bass_kernel_spmd

    global _CACHED_NC
    images = np.ascontiguousarray(np.asarray(inputs["images"], dtype=np.float32))
    maps = np.ascontiguousarray(np.asarray(inputs["maps"], dtype=np.float32))
    assert images.shape == (B, H, W, C) and maps.shape == (B, H, W, 1)

    if _CACHED_NC is None:
        _CACHED_NC = build_nc()
    nc = _CACHED_NC

    in_maps = [
        {"images": images[c * BC : (c + 1) * BC], "maps": maps[c * BC : (c + 1) * BC]}
        for c in range(N_CORES)
    ]
    res = run_bass_kernel_spmd(nc, in_maps, list(range(N_CORES)))
    return np.concatenate([r["out"] for r in res.results], axis=0)


# revision 15
# speedup vs baseline: 1.0851x; 1.0851x over previous
"""PixelUnshuffle(s=2) + avg-pool concat kernel for Trainium2, 8 NeuronCores.

Semantics (per image):
  out[i, j, 4c + 2b + a] = images[2i + a, 2j + b, c]   for c<3, a,b in {0,1}
  out[i, j, 12]          = mean of maps[2i:2i+2, 2j:2j+2]

Sharding: pure data-parallel over the batch dim (32 images -> 4 per core).

Per-core pipeline (per image):
  - one big HWDGE DMA: images rows -> SBUF, partition p holds input rows
    4p..4p+3 (contiguous 24.6 KB per partition)
  - one DMA: maps rows -> SBUF (same row grouping)
  - 4 strided tensor_copy ops (one per (a,b) phase) spread across
    DVE/ACT/POOL engines rearrange the image data into the output layout
  - 2 pool_avg ops (horizontal then vertical) compute the 2x2 mean into
    output channel 12
  - one big DMA: SBUF -> out (26.6 KB per partition)
The op is memory-bound; all engine work hides under the ~30 MB/core of
DMA traffic.
"""

import numpy as np

import concourse.bacc as bacc
import concourse.mybir as mybir
from concourse.tile import TileContext

N_CORES = 8
B, H, W, C = 32, 512, 512, 3
S = 2
BC = B // N_CORES  # images per core
HO, WO = H // S, W // S
K = C * S * S + 1  # 13 output channels
P = 128  # SBUF partitions

_FP = mybir.dt.float32


def build_nc(bc=BC, h=H, w=W, c=C, splits=2, bufs=4):
    """Build the SPMD Bass program for one core handling `bc` images.

    Each image is processed in `splits` row-chunks for finer DMA/compute
    pipelining (smaller tail stall, deeper prefetch with `bufs` slots).
    """
    ho, wo = h // S, w // S
    k = c * S * S + 1
    hh = h // splits  # input rows per chunk
    R = hh // P  # input rows per partition
    r = R // S  # output rows per partition
    assert R % S == 0 and hh % P == 0

    # Bacc (not raw Bass): its finalize() legalizes sync waits down to the
    # per-instruction caps walrus codegen enforces.
    nc = bacc.Bacc()
    images = nc.declare_dram_parameter("images", [bc, h, w, c], _FP, isOutput=False)
    maps = nc.declare_dram_parameter("maps", [bc, h, w, 1], _FP, isOutput=False)
    out = nc.declare_dram_parameter("out", [bc, ho, wo, k], _FP, isOutput=True)

    # engines for the 4 strided rearrange copies, one per (a, b) phase.
    # Keep the number of distinct engines writing out_tile small: walrus
    # codegen caps the sync-wait commands on the store DMA instruction.
    copy_engines = [nc.vector, nc.vector, nc.vector, nc.vector]

    with TileContext(nc) as tc:
        with tc.tile_pool(name="sbuf", bufs=bufs) as pool:
            for t in range(bc * splits):
                i, hp = divmod(t, splits)
                img_src = images[i][hp * hh : (hp + 1) * hh]
                map_src = maps[i][hp * hh : (hp + 1) * hh]
                out_dst = out[i][hp * (hh // S) : (hp + 1) * (hh // S)]

                img_tile = pool.tile([P, R * w * c], _FP, tag="img")
                map_tile = pool.tile([P, R * w], _FP, tag="map")
                out_tile = pool.tile([P, r * wo * k], _FP, tag="out")
                havg_tile = pool.tile([P, R * wo], _FP, tag="havg")

                # loads: partition p <- input rows R*p .. R*p+R-1 (contiguous)
                nc.sync.dma_start(
                    out=img_tile[:],
                    in_=img_src.rearrange("(p f) w c -> p (f w c)", p=P),
                )
                nc.sync.dma_start(
                    out=map_tile[:],
                    in_=map_src.rearrange("(p f) w c -> p (f w c)", p=P),
                )

                # image rearrange: for each (a, b), copy
                #   in : rows a::2 (within partition), cols b::2, all c
                #   out: channels (2b + a)::4 of every output pixel
                v_in = img_tile[:].rearrange("p (R w c) -> p R w c", R=R, w=w)
                v_out = out_tile[:].rearrange("p (r w k) -> p r w k", r=r, w=wo)
                idx = 0
                for a in range(S):
                    for b in range(S):
                        eng = copy_engines[idx % len(copy_engines)]
                        idx += 1
                        copy_fn = getattr(eng, "tensor_copy", None) or (
                            lambda out, in_: eng.copy(out, in_)
                        )
                        copy_fn(
                            out=v_out[:, :, :, 2 * b + a : k - 1 : 4],
                            in_=v_in[:, a::2, b::2, :],
                        )

                # maps 2x2 mean -> channel 12: horizontal add, vertical add,
                # then scale-by-0.25 into the output tile (all on gpsimd)
                m3 = map_tile[:].rearrange("p (R w) -> p R w", R=R)
                h_out = havg_tile[:].rearrange("p (R j) -> p R j", R=R)
                nc.gpsimd.tensor_add(
                    out=h_out, in0=m3[:, :, 0::2], in1=m3[:, :, 1::2]
                )
                h4 = havg_tile[:].rearrange("p (R j) -> p R j", R=R)
                vavg_tile = pool.tile([P, r * wo], _FP, tag="vavg")
                v4 = vavg_tile[:].rearrange("p (r j) -> p r j", r=r)
                nc.gpsimd.tensor_add(
                    out=v4, in0=h4[:, 0::2, :], in1=h4[:, 1::2, :]
                )
                # final scaled write on vector so out_tile has a single
                # producer engine (keeps the store DMA at one sync wait)
                nc.vector.tensor_scalar_mul(v_out[:, :, :, k - 1], v4, 0.25)

                # store: partition p -> output rows r*p .. r*p+r-1
                nc.sync.dma_start(
                    out=out_dst.rearrange("(p f) w k -> p (f w k)", p=P),
                    in_=out_tile[:],
                )
    nc.finalize()
    return nc


_CACHED_NC = None


def kernel(**inputs: np.ndarray) -> np.ndarray:
    from concourse.bass_utils import run_bass_kernel_spmd

    global _CACHED_NC
    images = np.ascontiguousarray(np.asarray(inputs["images"], dtype=np.float32))
    maps = np.ascontiguousarray(np.asarray(inputs["maps"], dtype=np.float32))
    assert images.shape == (B, H, W, C) and maps.shape == (B, H, W, 1)

    if _CACHED_NC is None:
        _CACHED_NC = build_nc()
    nc = _CACHED_NC

    in_maps = [
        {"images": images[c * BC : (c + 1) * BC], "maps": maps[c * BC : (c + 1) * BC]}
        for c in range(N_CORES)
    ]
    res = run_bass_kernel_spmd(nc, in_maps, list(range(N_CORES)))
    return np.concatenate([r["out"] for r in res.results], axis=0)


# revision 17
# speedup vs baseline: 1.0870x; 1.0017x over previous
"""PixelUnshuffle(s=2) + avg-pool concat kernel for Trainium2, 8 NeuronCores.

Semantics (per image):
  out[i, j, 4c + 2b + a] = images[2i + a, 2j + b, c]   for c<3, a,b in {0,1}
  out[i, j, 12]          = mean of maps[2i:2i+2, 2j:2j+2]

Sharding: pure data-parallel over the batch dim (32 images -> 4 per core).

Per-core pipeline:
  - maps are loaded once per image (1.05 MB DMA) with a 4D access pattern
    that puts rows {2p, 2p+1, 256+2p, 256+2p+1} on partition p, so both
    height-halves are partition-aligned with the compute chunks
  - images are loaded per height-half (1.57 MB DMA), partition p holding
    input rows {2p, 2p+1} of the half
  - compute + store run at quarter granularity (height-half x width-half):
    4 strided tensor_copy ops (DVE) rearrange the image data, two adds
    (GpSimd) + a scaled copy (DVE) produce the 2x2 map mean in channel 12,
    then a 0.85 MB store DMA writes the quarter
The op is memory-bound (~30 MB/core of DMA); all engine work hides under
the DMA stream, and the fine store granularity keeps the pipeline tail
short.
"""

import numpy as np

import concourse.bacc as bacc
import concourse.mybir as mybir
from concourse.tile import TileContext

N_CORES = 8
B, H, W, C = 32, 512, 512, 3
S = 2
BC = B // N_CORES  # images per core
HO, WO = H // S, W // S
K = C * S * S + 1  # 13 output channels
P = 128  # SBUF partitions

_FP = mybir.dt.float32


def build_nc(bc=BC, h=H, w=W, c=C, ws=2, img_bufs=3, map_bufs=2, out_bufs=4):
    """Build the SPMD Bass program for one core handling `bc` images.

    Each image is processed as 2 height-halves x `ws` width-chunks.
    """
    k = c * S * S + 1
    assert h == 4 * P  # height-half = 2 rows per partition
    hh = h // 2  # input rows per height-half
    wch = w // ws  # input cols per width-chunk
    woch = wch // S  # output cols per width-chunk
    assert w % ws == 0 and wch % S == 0

    # Bacc (not raw Bass): its finalize() legalizes sync waits down to the
    # per-instruction caps walrus codegen enforces.
    nc = bacc.Bacc()
    images = nc.declare_dram_parameter("images", [bc, h, w, c], _FP, isOutput=False)
    maps = nc.declare_dram_parameter("maps", [bc, h, w, 1], _FP, isOutput=False)
    out = nc.declare_dram_parameter("out", [bc, h // S, w // S, k], _FP, isOutput=True)

    with TileContext(nc) as tc:
        with (
            tc.tile_pool(name="pimg", bufs=img_bufs) as pimg,
            tc.tile_pool(name="pmap", bufs=map_bufs) as pmap,
            tc.tile_pool(name="pout", bufs=out_bufs) as pout,
            tc.tile_pool(name="psml", bufs=out_bufs) as psml,
        ):
            for i in range(bc):
                # whole image's maps; partition p gets rows
                # {2p, 2p+1, hh+2p, hh+2p+1} so both height-halves align
                map_tile = pmap.tile([P, 4 * w], _FP, tag="map")
                m4 = map_tile[:].rearrange("p (s rr w) -> p s rr w", s=2, rr=2)
                nc.sync.dma_start(
                    out=m4,
                    in_=maps[i].rearrange("(s p rr) w c -> p s rr (w c)", p=P, s=2),
                )

                for hp in range(2):
                    # height-half of the image; partition p <- rows {2p, 2p+1}
                    img_tile = pimg.tile([P, 2 * w * c], _FP, tag="img")
                    nc.sync.dma_start(
                        out=img_tile[:],
                        in_=images[i][hp * hh : (hp + 1) * hh].rearrange(
                            "(p f) w c -> p (f w c)", p=P
                        ),
                    )
                    v_in = img_tile[:].rearrange("p (R w c) -> p R w c", R=2, w=w)

                    for wq in range(ws):
                        out_tile = pout.tile([P, woch * k], _FP, tag="out")
                        v_out = out_tile[:].rearrange("p (w k) -> p w k", w=woch)

                        # image rearrange: channel 4c+2b+a <- rows a::2,
                        # cols b::2 of this quarter
                        for a in range(S):
                            for b in range(S):
                                nc.vector.tensor_copy(
                                    out=v_out[:, :, 2 * b + a : k - 1 : 4],
                                    in_=v_in[
                                        :,
                                        a::2,
                                        wq * wch + b : (wq + 1) * wch : 2,
                                        :,
                                    ],
                                )

                        # maps 2x2 mean -> channel 12: horizontal add,
                        # vertical add (GpSimd), scaled write (DVE - keeps
                        # out_tile single-producer-engine)
                        m_sub = m4[:, hp, :, wq * wch : (wq + 1) * wch]
                        havg = psml.tile([P, 2 * woch], _FP, tag="havg")
                        h2 = havg[:].rearrange("p (rr j) -> p rr j", rr=2)
                        nc.gpsimd.tensor_add(
                            out=h2, in0=m_sub[:, :, 0::2], in1=m_sub[:, :, 1::2]
                        )
                        vavg = psml.tile([P, woch], _FP, tag="vavg")
                        nc.gpsimd.tensor_add(
                            out=vavg[:], in0=h2[:, 0, :], in1=h2[:, 1, :]
                        )
                        nc.vector.tensor_scalar_mul(v_out[:, :, k - 1], vavg[:], 0.25)

                        # store this quarter: partition p -> output row p of
                        # the half, cols [wq*woch, (wq+1)*woch)
                        nc.sync.dma_start(
                            out=out[i][
                                hp * P : (hp + 1) * P,
                                wq * woch : (wq + 1) * woch,
                            ].rearrange("p w k -> p (w k)"),
                            in_=out_tile[:],
                        )
    nc.finalize()
    return nc


_CACHED_NC = None


def kernel(**inputs: np.ndarray) -> np.ndarray:
    from concourse.bass_utils import run_bass_kernel_spmd

    global _CACHED_NC
    images = np.ascontiguousarray(np.asarray(inputs["images"], dtype=np.float32))
    maps = np.ascontiguousarray(np.asarray(inputs["maps"], dtype=np.float32))
    assert images.shape == (B, H, W, C) and maps.shape == (B, H, W, 1)

    if _CACHED_NC is None:
        _CACHED_NC = build_nc()
    nc = _CACHED_NC

    in_maps = [
        {"images": images[c * BC : (c + 1) * BC], "maps": maps[c * BC : (c + 1) * BC]}
        for c in range(N_CORES)
    ]
    res = run_bass_kernel_spmd(nc, in_maps, list(range(N_CORES)))
    return np.concatenate([r["out"] for r in res.results], axis=0)


# revision 20
# speedup vs baseline: 1.0984x; 1.0105x over previous
"""PixelUnshuffle(s=2) + avg-pool concat kernel for Trainium2, 8 NeuronCores.

Semantics (per image):
  out[i, j, 4c + 2b + a] = images[2i + a, 2j + b, c]   for c<3, a,b in {0,1}
  out[i, j, 12]          = mean of maps[2i:2i+2, 2j:2j+2]

Sharding: pure data-parallel over the batch dim (32 images -> 4 per core).

Per-core pipeline:
  - maps are loaded once per image (1.05 MB DMA) with a 4D access pattern
    that puts rows {2p, 2p+1, 256+2p, 256+2p+1} on partition p, so both
    height-halves are partition-aligned with the compute chunks
  - images are loaded per height-half (1.57 MB DMA), partition p holding
    input rows {2p, 2p+1} of the half
  - compute + store run at quarter granularity (height-half x width-half):
    4 strided tensor_copy ops (DVE) rearrange the image data, two adds
    (GpSimd) + a scaled copy (DVE) produce the 2x2 map mean in channel 12,
    then a 0.85 MB store DMA writes the quarter
The op is memory-bound (~30 MB/core of DMA); all engine work hides under
the DMA stream, and the fine store granularity keeps the pipeline tail
short.
"""

import numpy as np

import concourse.bacc as bacc
import concourse.mybir as mybir
from concourse.tile import TileContext

N_CORES = 8
B, H, W, C = 32, 512, 512, 3
S = 2
BC = B // N_CORES  # images per core
HO, WO = H // S, W // S
K = C * S * S + 1  # 13 output channels
P = 128  # SBUF partitions

_FP = mybir.dt.float32


def build_nc(bc=BC, h=H, w=W, c=C, ws=2, img_bufs=3, map_bufs=2, out_bufs=4):
    """Build the SPMD Bass program for one core handling `bc` images.

    Each image is processed as 2 height-halves; full-width chunks except the
    final half, which is split into `ws` width-chunks to shorten the
    pipeline tail (last store only waits on a quarter's compute).
    """
    k = c * S * S + 1
    assert h == 4 * P  # height-half = 2 rows per partition
    hh = h // 2  # input rows per height-half
    assert w % ws == 0 and (w // ws) % S == 0

    # Bacc (not raw Bass): its finalize() legalizes sync waits down to the
    # per-instruction caps walrus codegen enforces.
    nc = bacc.Bacc()
    images = nc.declare_dram_parameter("images", [bc, h, w, c], _FP, isOutput=False)
    maps = nc.declare_dram_parameter("maps", [bc, h, w, 1], _FP, isOutput=False)
    out = nc.declare_dram_parameter("out", [bc, h // S, w // S, k], _FP, isOutput=True)

    with TileContext(nc) as tc:
        with (
            tc.tile_pool(name="pimg", bufs=img_bufs) as pimg,
            tc.tile_pool(name="pmap", bufs=map_bufs) as pmap,
            tc.tile_pool(name="pout", bufs=out_bufs) as pout,
            tc.tile_pool(name="psml", bufs=out_bufs) as psml,
        ):
            for i in range(bc):
                # whole image's maps; partition p gets rows
                # {2p, 2p+1, hh+2p, hh+2p+1} so both height-halves align
                map_tile = pmap.tile([P, 4 * w], _FP, tag="map")
                m4 = map_tile[:].rearrange("p (s rr w) -> p s rr w", s=2, rr=2)
                nc.sync.dma_start(
                    out=m4,
                    in_=maps[i].rearrange("(s p rr) w c -> p s rr (w c)", p=P, s=2),
                )

                for hp in range(2):
                    # height-half of the image; partition p <- rows {2p, 2p+1}
                    img_tile = pimg.tile([P, 2 * w * c], _FP, tag="img")
                    nc.sync.dma_start(
                        out=img_tile[:],
                        in_=images[i][hp * hh : (hp + 1) * hh].rearrange(
                            "(p f) w c -> p (f w c)", p=P
                        ),
                    )
                    v_in = img_tile[:].rearrange("p (R w c) -> p R w c", R=2, w=w)

                    cur_ws = ws if (i == bc - 1 and hp == 1) else 1
                    wch = w // cur_ws  # input cols per width-chunk
                    woch = wch // S  # output cols per width-chunk
                    for wq in range(cur_ws):
                        out_tile = pout.tile([P, woch * k], _FP, tag="out")
                        v_out = out_tile[:].rearrange("p (w k) -> p w k", w=woch)

                        # image rearrange: channel 4c+2b+a <- rows a::2,
                        # cols b::2 of this quarter
                        for a in range(S):
                            for b in range(S):
                                nc.vector.tensor_copy(
                                    out=v_out[:, :, 2 * b + a : k - 1 : 4],
                                    in_=v_in[
                                        :,
                                        a::2,
                                        wq * wch + b : (wq + 1) * wch : 2,
                                        :,
                                    ],
                                )

                        # maps 2x2 mean -> channel 12: horizontal add,
                        # vertical add (GpSimd), scaled write (DVE - keeps
                        # out_tile single-producer-engine)
                        m_sub = m4[:, hp, :, wq * wch : (wq + 1) * wch]
                        havg = psml.tile([P, 2 * woch], _FP, tag="havg")
                        h2 = havg[:].rearrange("p (rr j) -> p rr j", rr=2)
                        nc.gpsimd.tensor_add(
                            out=h2, in0=m_sub[:, :, 0::2], in1=m_sub[:, :, 1::2]
                        )
                        vavg = psml.tile([P, woch], _FP, tag="vavg")
                        nc.gpsimd.tensor_add(
                            out=vavg[:], in0=h2[:, 0, :], in1=h2[:, 1, :]
                        )
                        nc.vector.tensor_scalar_mul(v_out[:, :, k - 1], vavg[:], 0.25)

                        # store this chunk: partition p -> output row p of
                        # the half, cols [wq*woch, (wq+1)*woch). Issued on
                        # the ACT HW-DGE ring so stores (which wait on
                        # compute) never head-of-line-block the loads on
                        # the SP ring.
                        nc.scalar.dma_start(
                            out=out[i][
                                hp * P : (hp + 1) * P,
                                wq * woch : (wq + 1) * woch,
                            ].rearrange("p w k -> p (w k)"),
                            in_=out_tile[:],
                        )
    nc.finalize()
    return nc


_CACHED_NC = None


def kernel(**inputs: np.ndarray) -> np.ndarray:
    from concourse.bass_utils import run_bass_kernel_spmd

    global _CACHED_NC
    images = np.ascontiguousarray(np.asarray(inputs["images"], dtype=np.float32))
    maps = np.ascontiguousarray(np.asarray(inputs["maps"], dtype=np.float32))
    assert images.shape == (B, H, W, C) and maps.shape == (B, H, W, 1)

    if _CACHED_NC is None:
        _CACHED_NC = build_nc()
    nc = _CACHED_NC

    in_maps = [
        {"images": images[c * BC : (c + 1) * BC], "maps": maps[c * BC : (c + 1) * BC]}
        for c in range(N_CORES)
    ]
    res = run_bass_kernel_spmd(nc, in_maps, list(range(N_CORES)))
    return np.concatenate([r["out"] for r in res.results], axis=0)


# revision 21
# speedup vs baseline: 1.1024x; 1.0036x over previous
"""PixelUnshuffle(s=2) + avg-pool concat kernel for Trainium2, 8 NeuronCores.

Semantics (per image):
  out[i, j, 4c + 2b + a] = images[2i + a, 2j + b, c]   for c<3, a,b in {0,1}
  out[i, j, 12]          = mean of maps[2i:2i+2, 2j:2j+2]

Sharding: pure data-parallel over the batch dim (32 images -> 4 per core).

Per-core pipeline:
  - maps are loaded once per image (1.05 MB DMA) with a 4D access pattern
    that puts rows {2p, 2p+1, 256+2p, 256+2p+1} on partition p, so both
    height-halves are partition-aligned with the compute chunks
  - images are loaded per height-half (1.57 MB DMA), partition p holding
    input rows {2p, 2p+1} of the half
  - compute + store run at quarter granularity (height-half x width-half):
    4 strided tensor_copy ops (DVE) rearrange the image data, two adds
    (GpSimd) + a scaled copy (DVE) produce the 2x2 map mean in channel 12,
    then a 0.85 MB store DMA writes the quarter
The op is memory-bound (~30 MB/core of DMA); all engine work hides under
the DMA stream, and the fine store granularity keeps the pipeline tail
short.
"""

import numpy as np

import concourse.bacc as bacc
import concourse.mybir as mybir
from concourse.tile import TileContext

N_CORES = 8
B, H, W, C = 32, 512, 512, 3
S = 2
BC = B // N_CORES  # images per core
HO, WO = H // S, W // S
K = C * S * S + 1  # 13 output channels
P = 128  # SBUF partitions

_FP = mybir.dt.float32


def build_nc(bc=BC, h=H, w=W, c=C, ws=4, img_bufs=4, map_bufs=3, out_bufs=6):
    """Build the SPMD Bass program for one core handling `bc` images.

    Each image is processed as 2 height-halves; full-width chunks except the
    final half, which is split into `ws` width-chunks to shorten the
    pipeline tail (last store only waits on a quarter's compute).
    """
    k = c * S * S + 1
    assert h == 4 * P  # height-half = 2 rows per partition
    hh = h // 2  # input rows per height-half
    assert w % ws == 0 and (w // ws) % S == 0

    # Bacc (not raw Bass): its finalize() legalizes sync waits down to the
    # per-instruction caps walrus codegen enforces.
    nc = bacc.Bacc()
    images = nc.declare_dram_parameter("images", [bc, h, w, c], _FP, isOutput=False)
    maps = nc.declare_dram_parameter("maps", [bc, h, w, 1], _FP, isOutput=False)
    out = nc.declare_dram_parameter("out", [bc, h // S, w // S, k], _FP, isOutput=True)

    with TileContext(nc) as tc:
        with (
            tc.tile_pool(name="pimg", bufs=img_bufs) as pimg,
            tc.tile_pool(name="pmap", bufs=map_bufs) as pmap,
            tc.tile_pool(name="pout", bufs=out_bufs) as pout,
            tc.tile_pool(name="psml", bufs=out_bufs) as psml,
        ):
            for i in range(bc):
                # whole image's maps; partition p gets rows
                # {2p, 2p+1, hh+2p, hh+2p+1} so both height-halves align
                map_tile = pmap.tile([P, 4 * w], _FP, tag="map")
                m4 = map_tile[:].rearrange("p (s rr w) -> p s rr w", s=2, rr=2)
                nc.sync.dma_start(
                    out=m4,
                    in_=maps[i].rearrange("(s p rr) w c -> p s rr (w c)", p=P, s=2),
                )

                for hp in range(2):
                    # height-half of the image; partition p <- rows {2p, 2p+1}
                    img_tile = pimg.tile([P, 2 * w * c], _FP, tag="img")
                    nc.sync.dma_start(
                        out=img_tile[:],
                        in_=images[i][hp * hh : (hp + 1) * hh].rearrange(
                            "(p f) w c -> p (f w c)", p=P
                        ),
                    )
                    v_in = img_tile[:].rearrange("p (R w c) -> p R w c", R=2, w=w)

                    cur_ws = ws if (i == bc - 1 and hp == 1) else 1
                    wch = w // cur_ws  # input cols per width-chunk
                    woch = wch // S  # output cols per width-chunk
                    for wq in range(cur_ws):
                        out_tile = pout.tile([P, woch * k], _FP, tag="out")
                        v_out = out_tile[:].rearrange("p (w k) -> p w k", w=woch)

                        # image rearrange: channel 4c+2b+a <- rows a::2,
                        # cols b::2 of this quarter
                        for a in range(S):
                            for b in range(S):
                                nc.vector.tensor_copy(
                                    out=v_out[:, :, 2 * b + a : k - 1 : 4],
                                    in_=v_in[
                                        :,
                                        a::2,
                                        wq * wch + b : (wq + 1) * wch : 2,
                                        :,
                                    ],
                                )

                        # maps 2x2 mean -> channel 12: horizontal add,
                        # vertical add (GpSimd), scaled write (DVE - keeps
                        # out_tile single-producer-engine)
                        m_sub = m4[:, hp, :, wq * wch : (wq + 1) * wch]
                        havg = psml.tile([P, 2 * woch], _FP, tag="havg")
                        h2 = havg[:].rearrange("p (rr j) -> p rr j", rr=2)
                        nc.gpsimd.tensor_add(
                            out=h2, in0=m_sub[:, :, 0::2], in1=m_sub[:, :, 1::2]
                        )
                        vavg = psml.tile([P, woch], _FP, tag="vavg")
                        nc.gpsimd.tensor_add(
                            out=vavg[:], in0=h2[:, 0, :], in1=h2[:, 1, :]
                        )
                        nc.vector.tensor_scalar_mul(v_out[:, :, k - 1], vavg[:], 0.25)

                        # store this chunk: partition p -> output row p of
                        # the half, cols [wq*woch, (wq+1)*woch). Issued on
                        # the ACT HW-DGE ring so stores (which wait on
                        # compute) never head-of-line-block the loads on
                        # the SP ring.
                        nc.scalar.dma_start(
                            out=out[i][
                                hp * P : (hp + 1) * P,
                                wq * woch : (wq + 1) * woch,
                            ].rearrange("p w k -> p (w k)"),
                            in_=out_tile[:],
                        )
    nc.finalize()
    return nc


_CACHED_NC = None


def kernel(**inputs: np.ndarray) -> np.ndarray:
    from concourse.bass_utils import run_bass_kernel_spmd

    global _CACHED_NC
    images = np.ascontiguousarray(np.asarray(inputs["images"], dtype=np.float32))
    maps = np.ascontiguousarray(np.asarray(inputs["maps"], dtype=np.float32))
    assert images.shape == (B, H, W, C) and maps.shape == (B, H, W, 1)

    if _CACHED_NC is None:
        _CACHED_NC = build_nc()
    nc = _CACHED_NC

    in_maps = [
        {"images": images[c * BC : (c + 1) * BC], "maps": maps[c * BC : (c + 1) * BC]}
        for c in range(N_CORES)
    ]
    res = run_bass_kernel_spmd(nc, in_maps, list(range(N_CORES)))
    return np.concatenate([r["out"] for r in res.results], axis=0)


# revision 24
# speedup vs baseline: 1.1210x; 1.0169x over previous
"""PixelUnshuffle(s=2) + avg-pool concat kernel for Trainium2, 8 NeuronCores.

Semantics (per image):
  out[i, j, 4c + 2b + a] = images[2i + a, 2j + b, c]   for c<3, a,b in {0,1}
  out[i, j, 12]          = mean of maps[2i:2i+2, 2j:2j+2]

Sharding: pure data-parallel over the batch dim (32 images -> 4 per core).

Per-core pipeline:
  - maps are loaded once per image (1.05 MB DMA) with a 4D access pattern
    that puts rows {2p, 2p+1, 256+2p, 256+2p+1} on partition p, so both
    height-halves are partition-aligned with the compute chunks
  - images are loaded per height-half (1.57 MB DMA), partition p holding
    input rows {2p, 2p+1} of the half
  - compute + store run at quarter granularity (height-half x width-half):
    4 strided tensor_copy ops (DVE) rearrange the image data, two adds
    (GpSimd) + a scaled copy (DVE) produce the 2x2 map mean in channel 12,
    then a 0.85 MB store DMA writes the quarter
The op is memory-bound (~30 MB/core of DMA); all engine work hides under
the DMA stream, and the fine store granularity keeps the pipeline tail
short.
"""

import numpy as np

import concourse.bacc as bacc
import concourse.mybir as mybir
from concourse.tile import TileContext
from concourse.vector_clock import ScopedClock


class SlimExitTileContext(TileContext):
    """TileContext with a cheaper exit sequence.

    Stock Tile ends with drain + all-engine barrier + sem clears + second
    all-engine barrier (~4-6 us). The SP drain already waits on every proc's
    final tick (engines and DMA lanes), so it is sufficient to hand off
    SP -> Pool with a 2-engine barrier and let Pool clear the semaphores;
    Pool's halt orders the clears before NEFF completion, and the other
    engines simply halt after their last op.
    """

    def _drain_and_barrier(self, tick_clock, wait_clock):
        drain_inst = self.nc.sync.drain()
        wait_clock.add_sem_waits(
            drain_inst.ins, ScopedClock({None: tick_clock.global_clock})
        )
        self.nc.all_engine_barrier()
        popped = self.nc._tile_sem_poison_stack.pop()
        assert popped is self._sem_poison
        self.nc.clear_and_free_semaphores(list(self.sems.allocated().values()))
        # no trailing all-engine barrier: nothing follows this tile context,
        # Pool halts after its clears, and NEFF completion waits for all
        # engine halts - so the clears are ordered before the next execution

N_CORES = 8
B, H, W, C = 32, 512, 512, 3
S = 2
BC = B // N_CORES  # images per core
HO, WO = H // S, W // S
K = C * S * S + 1  # 13 output channels
P = 128  # SBUF partitions

_FP = mybir.dt.float32


def build_nc(bc=BC, h=H, w=W, c=C, ws=4, img_bufs=4, map_bufs=3, out_bufs=6):
    """Build the SPMD Bass program for one core handling `bc` images.

    Each image is processed as 2 height-halves; full-width chunks except the
    final half, which is split into `ws` width-chunks to shorten the
    pipeline tail (last store only waits on a quarter's compute).
    """
    k = c * S * S + 1
    assert h == 4 * P  # height-half = 2 rows per partition
    hh = h // 2  # input rows per height-half
    assert w % ws == 0 and (w // ws) % S == 0

    # Bacc (not raw Bass): its finalize() legalizes sync waits down to the
    # per-instruction caps walrus codegen enforces.
    nc = bacc.Bacc()
    images = nc.declare_dram_parameter("images", [bc, h, w, c], _FP, isOutput=False)
    maps = nc.declare_dram_parameter("maps", [bc, h, w, 1], _FP, isOutput=False)
    out = nc.declare_dram_parameter("out", [bc, h // S, w // S, k], _FP, isOutput=True)

    with SlimExitTileContext(nc) as tc:
        with (
            tc.tile_pool(name="pimg", bufs=img_bufs) as pimg,
            tc.tile_pool(name="pmap", bufs=map_bufs) as pmap,
            tc.tile_pool(name="pout", bufs=out_bufs) as pout,
            tc.tile_pool(name="psml", bufs=out_bufs) as psml,
        ):
            for i in range(bc):
                # whole image's maps; partition p gets rows
                # {2p, 2p+1, hh+2p, hh+2p+1} so both height-halves align
                map_tile = pmap.tile([P, 4 * w], _FP, tag="map")
                m4 = map_tile[:].rearrange("p (s rr w) -> p s rr w", s=2, rr=2)
                nc.sync.dma_start(
                    out=m4,
                    in_=maps[i].rearrange("(s p rr) w c -> p s rr (w c)", p=P, s=2),
                )

                for hp in range(2):
                    # height-half of the image; partition p <- rows {2p, 2p+1}
                    img_tile = pimg.tile([P, 2 * w * c], _FP, tag="img")
                    nc.sync.dma_start(
                        out=img_tile[:],
                        in_=images[i][hp * hh : (hp + 1) * hh].rearrange(
                            "(p f) w c -> p (f w c)", p=P
                        ),
                    )
                    v_in = img_tile[:].rearrange("p (R w c) -> p R w c", R=2, w=w)

                    cur_ws = ws if (i == bc - 1 and hp == 1) else 1
                    wch = w // cur_ws  # input cols per width-chunk
                    woch = wch // S  # output cols per width-chunk
                    for wq in range(cur_ws):
                        out_tile = pout.tile([P, woch * k], _FP, tag="out")
                        v_out = out_tile[:].rearrange("p (w k) -> p w k", w=woch)

                        # image rearrange: channel 4c+2b+a <- rows a::2,
                        # cols b::2 of this quarter
                        for a in range(S):
                            for b in range(S):
                                nc.vector.tensor_copy(
                                    out=v_out[:, :, 2 * b + a : k - 1 : 4],
                                    in_=v_in[
                                        :,
                                        a::2,
                                        wq * wch + b : (wq + 1) * wch : 2,
                                        :,
                                    ],
                                )

                        # maps 2x2 mean -> channel 12: horizontal add,
                        # vertical add (GpSimd), scaled write (DVE - keeps
                        # out_tile single-producer-engine)
                        m_sub = m4[:, hp, :, wq * wch : (wq + 1) * wch]
                        havg = psml.tile([P, 2 * woch], _FP, tag="havg")
                        h2 = havg[:].rearrange("p (rr j) -> p rr j", rr=2)
                        nc.gpsimd.tensor_add(
                            out=h2, in0=m_sub[:, :, 0::2], in1=m_sub[:, :, 1::2]
                        )
                        vavg = psml.tile([P, woch], _FP, tag="vavg")
                        nc.gpsimd.tensor_add(
                            out=vavg[:], in0=h2[:, 0, :], in1=h2[:, 1, :]
                        )
                        nc.vector.tensor_scalar_mul(v_out[:, :, k - 1], vavg[:], 0.25)

                        # store this chunk: partition p -> output row p of
                        # the half, cols [wq*woch, (wq+1)*woch). Issued on
                        # the ACT HW-DGE ring so stores (which wait on
                        # compute) never head-of-line-block the loads on
                        # the SP ring.
                        nc.scalar.dma_start(
                            out=out[i][
                                hp * P : (hp + 1) * P,
                                wq * woch : (wq + 1) * woch,
                            ].rearrange("p w k -> p (w k)"),
                            in_=out_tile[:],
                        )
    nc.finalize()
    return nc


_CACHED_NC = None


def kernel(**inputs: np.ndarray) -> np.ndarray:
    from concourse.bass_utils import run_bass_kernel_spmd

    global _CACHED_NC
    images = np.ascontiguousarray(np.asarray(inputs["images"], dtype=np.float32))
    maps = np.ascontiguousarray(np.asarray(inputs["maps"], dtype=np.float32))
    assert images.shape == (B, H, W, C) and maps.shape == (B, H, W, 1)

    if _CACHED_NC is None:
        _CACHED_NC = build_nc()
    nc = _CACHED_NC

    in_maps = [
        {"images": images[c * BC : (c + 1) * BC], "maps": maps[c * BC : (c + 1) * BC]}
        for c in range(N_CORES)
    ]
    res = run_bass_kernel_spmd(nc, in_maps, list(range(N_CORES)))
    return np.concatenate([r["out"] for r in res.results], axis=0)
